# revision 20
# baseline (speedup 1.0000x reference)
"""AffinityPropagate3 Trainium2 kernel.

Reference semantics (per batch sample, run on one NeuronCore):
    K = softmax(guided, axis=0)             # (9, H, W)
    mask = sign(sparse_depth)               # {0,1}
    x_{t+1} = mask*x0 + (1-mask) * sum_k K_k * shift_k(x_t),  16 steps,
    3x3 shifts with zero padding.

Sharding: pure data parallel, one batch sample per core (B=8, 8 cores).

On-chip layout: 120 partitions x 4 rows each (480 rows), each row padded
to 642 cols (one zero pad col per side), plus one halo row above/below
-> x buffer xa is [120, 6, 642] fp16.  All 9 stencil taps become
free-dim offset reads.  A second copy xb, shifted left by one element,
provides 4-byte-aligned views for the dw=0 taps so DVE 16-bit 2x mode
stays enabled.  Per iteration:
  - DVE: 9 fp16 tensor_tensor multiplies z_k = W_k * x[tap view]
  - TensorE: identity matmuls accumulate m0 + sum_k z_k into PSUM fp32
    (5 bank-aligned 512-col chunks)
  - ACT: PSUM -> xa and PSUM -> xb (fp32->fp16) writebacks
  - DMA: 4 partition-shifted SBUF->SBUF halo row refreshes
Softmax weights (x nomask / denom) are computed once up front; the
denominator is summed on TensorE as well.
"""

import sys

for _p in ("/opt/trn_rl_repo", "/root/.axon_site/_ro/trn_rl_repo"):
    if _p not in sys.path:
        sys.path.insert(0, _p)

import numpy as np

from concourse import bacc, mybir
from concourse import tile
import concourse.bass_utils as _bass_utils
from concourse.bass_utils import run_bass_kernel_spmd

# Keep matmul waits off InstLdweights (generate_event_semaphores already
# legalizes waits) so the dedup below only has to handle bare loads.
bacc.Bacc.move_matmul_waits_to_ldweights = lambda self: None


def dedup_ldweights(nc):
    """Every matmul in this kernel uses the same stationary identity
    matrix; drop all but the first InstLdweights (PE weights persist
    across matmuls).  An Ldweights carrying sync info becomes a NoOp so
    its waits/updates still fire."""
    for f in nc.m.functions:
        for bb in f.blocks:
            out = []
            seen_key = None
            changed = False
            for ins in bb.instructions:
                if type(ins).__name__ == "InstLdweights":
                    key = str(ins.ins[0])
                    if key == seen_key:
                        si = ins.sync_info
                        if si is not None and (si.on_wait or si.on_update):
                            out.append(
                                mybir.InstNoOp(
                                    name=ins.name + "-ldwn",
                                    engine=ins.engine,
                                    sync_info=si,
                                )
                            )
                        changed = True
                        continue
                    seen_key = key
                out.append(ins)
            if changed:
                bb.instructions[:] = out

B = 8
H, W = 480, 640
P = 120          # partitions used
RPP = 4          # rows per partition
WP = W + 2       # padded row width
NJ = RPP + 2     # row slots incl. halo
FLAT = RPP * W   # 2560 free elems per partition
CHUNK = 512      # matmul free-dim chunk (one PSUM bank of fp32)
NCH = FLAT // CHUNK
PROP_TIME = 16

FP32 = mybir.dt.float32
FP16 = mybir.dt.float16

# 3x3 tap order matching torch unfold channel order: k = ki*3 + kj,
# patches[k][h, w] = x[h + ki - 1, w + kj - 1].
TAPS = [(ki - 1, kj - 1) for ki in range(3) for kj in range(3)]
# Emission order: xa-owned-row taps first, then halo-dependent, then the
# dw=0 taps that read the shifted copy xb (ready last each iteration).
TAP_ORDER = [3, 5, 0, 2, 6, 8, 4, 1, 7]


def _rows_view(dram_ap):
    """DRAM [H, W] -> [P, RPP, W]."""
    return dram_ap.rearrange("(p r) w -> p r w", p=P)


def build_program(compile_=True):
    nc = bacc.Bacc("TRN2", target_bir_lowering=False, debug=False, num_devices=B)

    guided_d = nc.dram_tensor("guided", [9, H, W], FP32, kind="ExternalInput")
    x_d = nc.dram_tensor("x", [H, W], FP32, kind="ExternalInput")
    sparse_d = nc.dram_tensor("sparse_depth", [H, W], FP32, kind="ExternalInput")
    out_d = nc.dram_tensor("out", [H, W], FP32, kind="ExternalOutput")

    ident_d = nc.inline_tensor(np.eye(P, dtype=np.float16), name="ident_const")

    with tile.TileContext(nc) as tc:
        with (
            tc.tile_pool(name="persist", bufs=1) as persist,
            tc.tile_pool(name="work32", bufs=3) as work32,
            tc.tile_pool(name="zpool", bufs=6) as zpool,
            tc.tile_pool(name="psum", bufs=1, space="PSUM") as psump,
        ):
            # ---- persistent buffers ----
            xa = persist.tile([P, NJ, WP], FP16, tag="xa")
            xb = persist.tile([P, NJ, WP], FP16, tag="xb")
            wk = [
                persist.tile([P, FLAT], FP16, tag=f"wk{k}", name=f"wk{k}")
                for k in range(9)
            ]
            m0 = persist.tile([P, FLAT], FP16, tag="m0")
            nomask = persist.tile([P, FLAT], FP16, tag="nomask")
            rf16 = persist.tile([P, FLAT], FP16, tag="rf16")
            ident = persist.tile([P, P], FP16, tag="ident")
            den32 = persist.tile([P, FLAT], FP32, tag="den32")
            r32 = persist.tile([P, FLAT], FP32, tag="r32")
            stag = persist.tile([P, RPP, W], FP32, tag="stag")

            psum = psump.tile([P, FLAT], FP32, tag="psum")

            nc.vector.memset(xa[:], 0.0)
            nc.vector.memset(xb[:], 0.0)
            nc.sync.dma_start(out=ident[:], in_=ident_d[:])

            # ---- x load (cast fp32->fp16 via SWDGE) ----
            xd = _rows_view(x_d[:])
            nc.gpsimd.dma_start(out=xa[:, 1 : 1 + RPP, 1 : 1 + W], in_=xd)
            nc.gpsimd.dma_start(
                out=xa[1:P, 0:1, 1 : 1 + W], in_=xd[0 : P - 1, 3:4, :]
            )
            nc.gpsimd.dma_start(
                out=xa[0 : P - 1, 5:6, 1 : 1 + W], in_=xd[1:P, 0:1, :]
            )
            # xb = xa shifted left one element (flat)
            nflat = NJ * WP
            xaf = xa.rearrange("p a b -> p (a b)")
            xbf = xb.rearrange("p a b -> p (a b)")
            nc.vector.tensor_copy(
                out=xbf[:, 0 : nflat - 1], in_=xaf[:, 1:nflat]
            )

            # ---- masks ----
            sp = work32.tile([P, RPP, W], FP32, tag="g32")
            nc.sync.dma_start(out=sp[:], in_=_rows_view(sparse_d[:]))
            nc.vector.tensor_scalar(
                out=nomask[:], in0=sp.rearrange("p a b -> p (a b)")[:], scalar1=0.0,
                scalar2=None, op0=mybir.AluOpType.is_equal,
            )
            xv = xa[:, 1 : 1 + RPP, 1 : 1 + W]
            m0v = m0.rearrange("p (a b) -> p a b", a=RPP)
            nc.vector.tensor_tensor(
                out=m0v[:], in0=nomask.rearrange("p (a b) -> p a b", a=RPP)[:], in1=xv,
                op=mybir.AluOpType.mult,
            )
            nc.vector.tensor_tensor(
                out=m0v[:], in0=xv, in1=m0v[:], op=mybir.AluOpType.subtract
            )

            # ---- softmax: exp on ACT, denominator summed on TensorE ----
            for k in range(9):
                g32 = work32.tile([P, RPP, W], FP32, tag="g32")
                nc.sync.dma_start(out=g32[:], in_=_rows_view(guided_d[k]))
                nc.scalar.activation(
                    out=wk[k][:], in_=g32.rearrange("p a b -> p (a b)")[:],
                    func=mybir.ActivationFunctionType.Exp,
                )
                for c in range(NCH):
                    nc.tensor.matmul(
                        out=psum[:, c * CHUNK : (c + 1) * CHUNK],
                        lhsT=ident[:],
                        rhs=wk[k][:, c * CHUNK : (c + 1) * CHUNK],
                        start=(k == 0),
                        stop=(k == 8),
                    )
            nc.scalar.copy(out=den32[:], in_=psum[:])
            nc.vector.reciprocal_approx_fast(out=r32[:], in_=den32[:])
            nc.vector.tensor_tensor(
                out=rf16[:], in0=r32[:], in1=nomask[:], op=mybir.AluOpType.mult
            )
            for k in range(9):
                nc.vector.tensor_tensor(
                    out=wk[k][:], in0=wk[k][:], in1=rf16[:],
                    op=mybir.AluOpType.mult,
                )

            # ---- 16 propagation iterations ----
            for t in range(PROP_TIME):
                # seed psum with m0 via ACT (frees TensorE of 5 matmuls)
                nc.scalar.copy(out=psum[:], in_=m0[:])
                for idx, k in enumerate(TAP_ORDER):
                    dh, dw = TAPS[k]
                    if dw == 0:
                        # odd-offset view -> read the shifted copy xb
                        xsrc = xb[:, 1 + dh : 1 + dh + RPP, 0:W]
                    else:
                        xsrc = xa[:, 1 + dh : 1 + dh + RPP, 1 + dw : 1 + dw + W]
                    z = zpool.tile([P, RPP, W], FP16, tag="z")
                    nc.vector.tensor_tensor(
                        out=z[:], in0=wk[k].rearrange("p (a b) -> p a b", a=RPP)[:], in1=xsrc,
                        op=mybir.AluOpType.mult,
                    )
                    zf = z.rearrange("p a b -> p (a b)")
                    for c in range(NCH):
                        nc.tensor.matmul(
                            out=psum[:, c * CHUNK : (c + 1) * CHUNK],
                            lhsT=ident[:],
                            rhs=zf[:, c * CHUNK : (c + 1) * CHUNK],
                            start=False,
                            stop=(idx == 8),
                            skip_group_check=True,
                        )

                pv = psum.rearrange("p (a b) -> p a b", a=RPP)
                if t < PROP_TIME - 1:
                    # writebacks: psum rows (2,640)-pairs -> xa and xb
                    nc.scalar.copy(out=xa[:, 1:3, 1 : 1 + W], in_=pv[:, 0:2])
                    nc.scalar.copy(out=xa[:, 3:5, 1 : 1 + W], in_=pv[:, 2:4])
                    nc.scalar.copy(out=xb[:, 1:3, 0:W], in_=pv[:, 0:2])
                    nc.scalar.copy(out=xb[:, 3:5, 0:W], in_=pv[:, 2:4])
                    # halo refreshes (partition-shifted SBUF->SBUF)
                    nc.sync.dma_start(
                        out=xa[1:P, 0:1, 1 : 1 + W], in_=xa[0 : P - 1, 4:5, 1 : 1 + W]
                    )
                    nc.sync.dma_start(
                        out=xa[0 : P - 1, 5:6, 1 : 1 + W], in_=xa[1:P, 1:2, 1 : 1 + W]
                    )
                    nc.sync.dma_start(
                        out=xb[1:P, 0:1, 0:W], in_=xa[0 : P - 1, 4:5, 1 : 1 + W]
                    )
                    nc.sync.dma_start(
                        out=xb[0 : P - 1, 5:6, 0:W], in_=xa[1:P, 1:2, 1 : 1 + W]
                    )
                else:
                    nc.scalar.copy(out=stag[:, 0:2], in_=pv[:, 0:2])
                    nc.scalar.copy(out=stag[:, 2:4], in_=pv[:, 2:4])

            nc.sync.dma_start(out=_rows_view(out_d[:]), in_=stag[:])

    if compile_:
        nc.compile()
        dedup_ldweights(nc)
    return nc


_CACHED_NC = None


def _get_nc():
    global _CACHED_NC
    if _CACHED_NC is None:
        _CACHED_NC = build_program()
    return _CACHED_NC


def kernel(guided, x, sparse_depth, _trace=False, _trace_kwargs=None):
    guided = np.ascontiguousarray(guided, dtype=np.float32)
    x = np.ascontiguousarray(x, dtype=np.float32)
    sparse_depth = np.ascontiguousarray(sparse_depth, dtype=np.float32)
    assert guided.shape == (B, 9, H, W)

    nc = _get_nc()
    in_maps = [
        {
            "guided": guided[b],
            "x": x[b, 0],
            "sparse_depth": sparse_depth[b, 0],
        }
        for b in range(B)
    ]
    res = run_bass_kernel_spmd(
        nc, in_maps, list(range(B)), trace=_trace, **(_trace_kwargs or {})
    )
    out = np.stack([res.results[b]["out"] for b in range(B)])[:, None]
    if _trace:
        return out.astype(np.float32), res
    return out.astype(np.float32)
